# revision 43
# baseline (speedup 1.0000x reference)
"""ChildSum TreeLSTM (N=8192 nodes, 4-ary static heap tree, H=256, D=300) on 8 trn2 NeuronCores.

Strategy
--------
The tree is static: node i's children are 4i+1..4i+4 (clipped at N). The reverse
scan (children before parents) is equivalent to processing the tree level by
level, bottom-up; nodes within a level are independent, so each level is a
batched LSTM cell (matmuls + elementwise).

Sharding: the 256 level-4 subtrees are partitioned across the 8 cores (balanced
by the number of *internal* level-6 descendants). Each core processes its
forest fully locally; cores ship their L6-head and L5-bulk (og, c) states; the
tiny top of the tree (levels 4..0 plus L5-head) runs on the host in numpy.

On-device layout: feature dim on SBUF partitions (256 features = 2 halves of
128), nodes along the free axis, per-core column order
    [L7 (384, k-major by L6-int parent) | L6-leaf (416, k-major by L5-bulk
     parent) | L6-int (96) | L5 (128)]
Key perf choices:
  * fp8 e4m3 embeddings/Wx with DoubleRow matmuls: the 301-row contraction is
    zero-padded to four 128-row k-subtiles (subtile 3 memset on device, never
    transferred) running as two DoubleRow passes — measured fastest fp8 mode
    (fp8 without DoubleRow gets no fast-weight-load and is slower).
  * act-drains merge both feature halves per gate into one ACTIVATE
    ([128,2,cols] psum spans 2 banks) to amortize the ~350-cycle fixed cost;
    internal columns are never act-drained (phase 2 recomputes their gates),
    only copy-drained raw into GXI/GX3. The scalar activation queue is the
    end-to-end critical resource (~12us back-to-back).
  * the scalar queue (whose runtime prologue ends ~1.2us before the others')
    issues the first-needed weight DMA ahead of its ACT_TABLE_LOAD, pulling
    the whole pipeline ~2.3us earlier.
  * k-major child layout: the 4 children of the ip nodes of a chunk live in 4
    contiguous ip-wide blocks, so child-h sums and f*c sums are 3 contiguous
    bf16 adds (DVE 2x mode) instead of strided reduces.
  * phase-2 elementwise is split across engines: c1/c4 chains on gpsimd,
    c3/c5 (the critical tail) on vector; scalar runs all LUT activations
    back-to-back in dependency order.
  * phase-2 gate PSUM preloaded with gx via identity matmuls (descale folded
    into eye), wh @ hs accumulates on top; i&o share one sigmoid act; the
    f-gate bias is broadcast-preloaded per child block through the PE.
  * bf16 recurrence weights and h/c state; biases folded into an extra
    ones-row of the x-side matmul so pad columns self-compute to h = c = 0.
  * a short warm-up matmul stream during the input-DMA window ramps the PE
    out of its low p-state before the real x-side projections begin.
"""

import numpy as np
import ml_dtypes

BF16 = ml_dtypes.bfloat16
FP8 = ml_dtypes.float8_e4m3fn

N = 8192
H = 256
D = 300
K = 4
OUT = 4
NCORES = 8
IPMAX = 96          # max internal level-6 nodes per core
L7P = 384           # level-7 columns per core (4 * IPMAX, k-major)
L6L = 416           # level-6 leaf columns (4 * 104, k-major)
NL5B = 104          # L5 bulk nodes per core (l5[24:128])
XCOLS = L7P + L6L + IPMAX + NL5B  # 1000: [L7 | L6leaf | L6int | L5bulk]
KUSE = 301          # contraction rows (300 emb + 1 ones)
XS = 32.0           # fp8 quantization scale for embeddings
WS = 8.0            # fp8 quantization scale for Wx
DESCALE = 1.0 / (XS * WS)

GATE_MAP = [0, 2, 3, 1]  # our gate order [i, o, u, f] -> reference gate indices

F32 = np.float32

# phase-1 column ranges (each one act-drain per gate; range 1's last 200
# columns are the internal window, copy-drained only)
_RANGES = [(0, 512), (512, 1000)]
INTW = 200          # internal window = cols 800:1000 = [L6int 96 | L5bulk 104]


def _build_plan():
    """Assign the 256 level-4 subtrees to 8 cores; build per-core column maps."""
    full = list(range(85, 127))                               # 42 subtrees
    lights = list(range(128, 341))                            # 213 subtrees
    heavy_counts = [6, 6, 5, 5, 5, 5, 5, 5]                   # sums to 42
    light_counts = [26, 26, 26, 27, 27, 27, 27, 27]           # sums to 213
    cores = []
    hpos = 0
    lpos = 0
    for c in range(NCORES):
        hs = full[hpos:hpos + heavy_counts[c]]
        hpos += heavy_counts[c]
        if c == 2:
            hs = hs + [127]
        ls = lights[lpos:lpos + light_counts[c]]
        lpos += light_counts[c]
        cores.append(sorted(hs + ls))
    all_l4 = sorted(u for cs in cores for u in cs)
    assert all_l4 == list(range(85, 341)), "L4 assignment must partition [85, 341)"

    plan = []
    for c in range(NCORES):
        l4 = cores[c]
        assert len(l4) == 32
        l5 = [4 * u + 1 + k for u in l4 for k in range(K)]        # 128, ascending
        l6 = [4 * v + 1 + k for v in l5 for k in range(K)]        # 512, ascending
        wc = sum(1 for x in l6 if x < 2048)
        assert wc <= IPMAX
        l6head = l6[:IPMAX]            # children of l5[0:24] (first wc internal)
        l6leaf = l6[IPMAX:]            # children of l5[24:128], all leaves
        # L7 section, k-major: col k*96 + j = child k of l6head[j]
        l7 = [-1] * L7P
        for j in range(IPMAX):
            if j < wc:
                for k in range(K):
                    ch = 4 * l6head[j] + 1 + k
                    if ch < N:
                        l7[k * IPMAX + j] = ch
        # L6-leaf section, k-major: col k*104 + j = child k of l5[24+j]
        l6l = [0] * L6L
        for j in range(NL5B):
            for k in range(K):
                l6l[k * NL5B + j] = 4 * l5[24 + j] + 1 + k
        cols = np.array(l7 + l6l + l6head + l5[24:], dtype=np.int64)
        assert cols.shape == (XCOLS,)
        plan.append((cols, wc, np.array(l5, dtype=np.int64)))
    return plan


_PLAN = _build_plan()


def _static_tree():
    idx = np.arange(N)[:, None] * K + 1 + np.arange(K)[None, :]
    mask = (idx < N).astype(F32)
    idx = np.where(idx < N, idx, 0).astype(np.int32)
    return idx, mask


_STATIC_IDX, _STATIC_MASK = _static_tree()


def _pack_weights(Wx, bx, Wh, bh):
    """Pack to partition-major HBM layouts.

    wx3  [128, 4, 1024] fp8e4m3 (x WS): wx3[p, k, 256*g + j] = Wx[rg][j, 128k+p]
                               row 300 (k=2, p=44) holds (bx+bh) * WS; the
                               4th k-subtile is all-zero (memset on device,
                               never transferred).
    wh2  [128, 2, 768]  bf16 : i/o/u recurrence weights, transposed.
    whf2 [128, 2, 256]  bf16 : forget recurrence weights, transposed.
    """
    wx = np.zeros((512, 4 * H), dtype=F32)
    for g, rg in enumerate(GATE_MAP):
        wx[:D, H * g:H * (g + 1)] = np.asarray(Wx[rg], dtype=F32).T
        wx[D, H * g:H * (g + 1)] = np.asarray(bx[rg], dtype=F32) + np.asarray(bh[rg], dtype=F32)
    wx3 = np.ascontiguousarray(
        (wx * WS).reshape(4, 128, 4 * H).transpose(1, 0, 2)).astype(FP8)
    wh = np.zeros((H, 3 * H), dtype=F32)
    for g, rg in enumerate([0, 2, 3]):  # i, o, u
        wh[:, H * g:H * (g + 1)] = np.asarray(Wh[rg], dtype=F32).T
    wh2 = np.ascontiguousarray(wh.reshape(2, 128, 3 * H).transpose(1, 0, 2)).astype(BF16)
    whf = np.asarray(Wh[1], dtype=F32).T
    whf2 = np.ascontiguousarray(whf.reshape(2, 128, H).transpose(1, 0, 2)).astype(BF16)
    return wx3, wh2, whf2


def _pack_xt(xs, emb_table):
    """Per-core transposed embeddings, partition-major: [128, 4, XCOLS] fp8."""
    X = np.asarray(emb_table, dtype=F32)[np.asarray(xs)]
    xts = []
    for cols, _, _ in _PLAN:
        xt = np.zeros((512, XCOLS), dtype=F32)
        real = cols >= 0
        xt[:D, real] = X[cols[real]].T
        xt[D, real] = 1.0
        xt4 = np.ascontiguousarray(
            (xt * XS).reshape(4, 128, XCOLS).transpose(1, 0, 2)).astype(FP8)
        xts.append(xt4)
    return xts


def _sigmoid(x):
    return (1.0 / (1.0 + np.exp(-x))).astype(F32)


def _host_top(Hbuf, Cbuf, xs, emb_table, Wx, bx, Wh, bh):
    """Compute tree levels 4..0 (nodes 0..340) on the host, numpy fp32."""
    Wx = np.asarray(Wx, dtype=F32)
    bx = np.asarray(bx, dtype=F32)
    Wh = np.asarray(Wh, dtype=F32)
    bh = np.asarray(bh, dtype=F32)
    emb = np.asarray(emb_table, dtype=F32)
    xs = np.asarray(xs)
    for lo, hi in [(85, 341), (21, 85), (5, 21), (1, 5), (0, 1)]:
        ids = np.arange(lo, hi)
        Xl = emb[xs[ids]]                                   # [n, D]
        gx = np.einsum('ghd,nd->ngh', Wx, Xl).astype(F32) + bx
        cidx = ids[:, None] * K + 1 + np.arange(K)[None, :]  # all valid (< 341)
        Hc = Hbuf[cidx]
        Cc = Cbuf[cidx]
        hs = Hc.sum(1)
        ig = _sigmoid(gx[:, 0] + hs @ Wh[0].T + bh[0])
        og = _sigmoid(gx[:, 2] + hs @ Wh[2].T + bh[2])
        ug = np.tanh(gx[:, 3] + hs @ Wh[3].T + bh[3]).astype(F32)
        f = _sigmoid(gx[:, 1][:, None, :] + Hc @ Wh[1].T + bh[1])
        cc = ig * ug + (f * Cc).sum(1)
        hh = og * np.tanh(cc).astype(F32)
        Hbuf[ids] = hh
        Cbuf[ids] = cc
    return Hbuf[0]


def _log_softmax(x):
    m = np.max(x)
    e = np.exp(x - m)
    return (x - m - np.log(e.sum())).astype(F32)


# ----------------------------------------------------------------------------
# Bass device program
# ----------------------------------------------------------------------------

_COMPILED = None


def _build_device_program():
    import contextlib

    import concourse.bacc as bacc
    import concourse.tile as tile
    import concourse.mybir as mybir

    f32 = mybir.dt.float32
    bf16 = mybir.dt.bfloat16
    fp8 = mybir.dt.float8e4
    Sig = mybir.ActivationFunctionType.Sigmoid
    Tanh = mybir.ActivationFunctionType.Tanh

    nc = bacc.Bacc("TRN2", target_bir_lowering=False, debug=False,
                   num_devices=NCORES)

    DRow = mybir.MatmulPerfMode.DoubleRow
    xt_d = nc.dram_tensor("xt", [128, 4, XCOLS], fp8, kind="ExternalInput")
    wx_d = nc.dram_tensor("wx", [128, 4, 4 * H], fp8, kind="ExternalInput")
    wh_d = nc.dram_tensor("wh", [128, 2, 3 * H], bf16, kind="ExternalInput")
    whf_d = nc.dram_tensor("whf", [128, 2, H], bf16, kind="ExternalInput")
    eye_d = nc.dram_tensor("eye", [128, 128], bf16, kind="ExternalInput")
    out_h_d = nc.dram_tensor("out_h", [128, 2, NL5B], bf16, kind="ExternalOutput")
    out_c_d = nc.dram_tensor("out_c", [128, 2, NL5B], bf16, kind="ExternalOutput")
    out_h6_d = nc.dram_tensor("out_h6", [128, 2, IPMAX], bf16, kind="ExternalOutput")
    out_c6_d = nc.dram_tensor("out_c6", [128, 2, IPMAX], bf16, kind="ExternalOutput")

    GFUNC = [Sig, Sig, Tanh]   # activation per gate i, o, u

    with tile.TileContext(nc) as tc:
        with contextlib.ExitStack() as ctx:
            inp = ctx.enter_context(tc.tile_pool(name="inp", bufs=1))
            st = ctx.enter_context(tc.tile_pool(name="state", bufs=1))
            wk = ctx.enter_context(tc.tile_pool(name="work", bufs=2))

            # --- input SBUF tiles (single tile per tensor; sliced DMAs)
            xt_s = inp.tile([128, 4, XCOLS], fp8, tag="xt", name="xt")
            wx_s = inp.tile([128, 4, 4 * H], fp8, tag="wx", name="wx")
            wh_s = inp.tile([128, 2, 3 * H], bf16, tag="wh", name="wh")
            whf_s = inp.tile([128, 2, H], bf16, tag="whf", name="whf")
            eye_s = inp.tile([128, 128], bf16, tag="eye", name="eye")

            # priority DMAs. Only sync / scalar / gpsimd queues can issue
            # DMAs. The scalar queue's prologue finishes ~1.2us before the
            # others, so it issues the first-needed weight slice (gate i),
            # ahead of its ACT_TABLE_LOAD; its act stream starts much later.
            a0, b0 = _RANGES[0]
            nc.scalar.dma_start(out=wx_s[:, 0:3, 0:256], in_=wx_d[:, 0:3, 0:256])
            nc.sync.dma_start(out=xt_s[:, 0:3, a0:b0], in_=xt_d[:, 0:3, a0:b0])
            nc.gpsimd.dma_start(out=wx_s[:, 0:3, 256:768],
                                in_=wx_d[:, 0:3, 256:768])
            nc.sync.dma_start(out=eye_s[:], in_=eye_d[:])
            a1, b1 = _RANGES[1]
            nc.sync.dma_start(out=xt_s[:, 0:3, a1:b1], in_=xt_d[:, 0:3, a1:b1])
            nc.gpsimd.dma_start(out=wx_s[:, 0:3, 768:1024],
                                in_=wx_d[:, 0:3, 768:1024])
            nc.gpsimd.dma_start(out=whf_s[:], in_=whf_d[:])
            nc.sync.dma_start(out=wh_s[:], in_=wh_d[:])

            # --- persistent state + gate tiles
            # activated gates for the 800 leaf columns (L7 + L6-leaf)
            G = [st.tile([128, 2, 800], bf16, tag=f"g{g}", name=f"g{g}")
                 for g in range(3)]
            # raw (fp8-scaled) gx for the 224 internal cols, bf16; consumed
            # via identity-matmul psum preloads which fold in the descale
            GXI = [st.tile([128, 2, INTW], bf16, tag=f"gxi{g}", name=f"gxi{g}")
                   for g in range(3)]
            GX3 = st.tile([128, 2, INTW], bf16, tag="gx3", name="gx3")
            # leaf states
            SH7 = st.tile([128, 2, L7P], bf16, tag="sh7", name="sh7")
            SC7 = st.tile([128, 2, L7P], bf16, tag="sc7", name="sc7")
            SH6 = st.tile([128, 2, L6L], bf16, tag="sh6", name="sh6")
            SC6 = st.tile([128, 2, L6L], bf16, tag="sc6", name="sc6")
            # internal-chunk outputs (i&o sigmoid-merged tile + u tile)
            GIO6 = st.tile([128, 2, 2, IPMAX], bf16, tag="gio6", name="gio6")
            GU6 = st.tile([128, 2, IPMAX], bf16, tag="gu6", name="gu6")
            C6I = st.tile([128, 2, IPMAX], bf16, tag="c6i", name="c6i")
            GIO5 = st.tile([128, 2, 2, NL5B], bf16, tag="gio5", name="gio5")
            GU5 = st.tile([128, 2, NL5B], bf16, tag="gu5", name="gu5")
            C5V = st.tile([128, 2, NL5B], bf16, tag="c5v", name="c5v")

            # --- PE warm-up: ramp the tensor engine to full clock during
            # the input-DMA window (the p-state ramp needs ~3us of
            # continuous execution; these matmuls depend only on a memset).
            wz = wk.tile([128, 512], bf16, tag="wz", name="wz")
            nc.vector.memset(wz[:], 0.0)
            # zero 4th k-subtiles (read by the second DoubleRow pass); on
            # vector, which is otherwise idle until the first GXI cast
            nc.vector.memset(xt_s[:, 3, :], 0.0)
            nc.vector.memset(wx_s[:, 3, :], 0.0)
            with tc.tile_pool(name="psum0", bufs=1, space="PSUM") as ps0:
                for w in range(6):
                    pw = ps0.tile([128, 512], f32, tag="pw", bufs=2,
                                  name=f"pw{w}")
                    nc.tensor.matmul(pw[:], wz[:, 0:128], wz[:],
                                     start=True, stop=True)

            with nc.allow_low_precision("bf16 gates/h-state within 2e-2 tol"):
                # --- phase 1: x-side projections, range-major sweep.
                # Three plain fp8 k-subtile passes per (range, gate, phi);
                # one phi-merged act-drain per (range, gate); internal window
                # copy-drained raw into GXI; f computed only on the internal
                # window into GX3.
                ps1_cm = tc.tile_pool(name="psum1", bufs=1, space="PSUM")
                ps = ps1_cm.__enter__()
                for ri, (a, b) in enumerate(_RANGES):
                    for g in (0, 2, 1):          # i, u first: c = i*u unblocks
                        pt = ps.tile([128, 2, 512], f32, tag="gx", bufs=3,
                                     name=f"pgx{ri}_{g}")
                        for phi in range(2):
                            col = 256 * g + 128 * phi
                            # fp8 DoubleRow pairs: contraction padded to 4
                            # 128-row k-subtiles (subtile 3 all-zero)
                            nc.tensor.matmul(
                                pt[:, phi, 0:b - a],
                                wx_s[:, 0:2, col:col + 128],
                                xt_s[:, 0:2, a:b],
                                start=True, stop=False, perf_mode=DRow)
                            nc.tensor.matmul(
                                pt[:, phi, 0:b - a],
                                wx_s[:, 2:4, col:col + 128],
                                xt_s[:, 2:4, a:b],
                                start=False, stop=True, perf_mode=DRow)
                        if ri == 0:
                            # all 512 cols are leaves
                            nc.scalar.activation(
                                G[g][:, :, a:b], pt[:], GFUNC[g],
                                scale=DESCALE)
                        else:
                            # leaf part 512:800 act-drained; internal window
                            # copy-drained raw on gpsimd (vector is busy
                            # with the leaf c/h chains here)
                            nc.scalar.activation(
                                G[g][:, :, a:800], pt[:, :, 0:800 - a],
                                GFUNC[g], scale=DESCALE)
                            nc.vector.tensor_copy(
                                GXI[g][:], pt[:, :, 800 - a:b - a])
                        # c1 / c3 leaf elementwise, interleaved at the right
                        # dependency points (c=i*u after u-act; h after o-act)
                        if ri == 0 and g == 2:
                            # SC7 = i*u over L7 cols
                            nc.vector.tensor_mul(
                                SC7[:], G[0][:, :, 0:L7P], G[2][:, :, 0:L7P])
                        if ri == 0 and g == 1:
                            TC7 = wk.tile([128, 2, L7P], bf16, tag="tc7",
                                          name="tc7")
                            nc.scalar.activation(TC7[:], SC7[:], Tanh)
                            nc.vector.tensor_mul(SH7[:], G[1][:, :, 0:L7P],
                                                 TC7[:])
                        if ri == 1 and g == 2:
                            nc.vector.tensor_mul(
                                SC6[:], G[0][:, :, L7P:800],
                                G[2][:, :, L7P:800])
                ps1 = ps

                # f-gate x-projection on the internal window only
                gxf = ps1.tile([128, 2, 512], f32, tag="gxf", name="gxf")
                for phi in range(2):
                    col = 768 + 128 * phi
                    nc.tensor.matmul(
                        gxf[:, phi, 0:INTW], wx_s[:, 0:2, col:col + 128],
                        xt_s[:, 0:2, 800:1000], start=True, stop=False,
                        perf_mode=DRow)
                    nc.tensor.matmul(
                        gxf[:, phi, 0:INTW], wx_s[:, 2:4, col:col + 128],
                        xt_s[:, 2:4, 800:1000], start=False, stop=True,
                        perf_mode=DRow)
                nc.vector.tensor_copy(GX3[:], gxf[:, :, 0:INTW])

                # hs6 = sum of the 4 child blocks (k-major: contiguous); on
                # gpsimd — small SBUF-only adds, frees the congested vector
                # queue between the GXI casts and the c3 tail
                hs6 = wk.tile([128, 2, IPMAX], bf16, tag="hs", name="hs6")
                t16 = wk.tile([128, 2, IPMAX], bf16, tag="t1", name="t16")
                nc.gpsimd.tensor_add(t16[:], SH7[:, :, 0:IPMAX],
                                     SH7[:, :, IPMAX:2 * IPMAX])
                nc.gpsimd.tensor_add(hs6[:], SH7[:, :, 2 * IPMAX:3 * IPMAX],
                                     SH7[:, :, 3 * IPMAX:4 * IPMAX])
                nc.gpsimd.tensor_add(hs6[:], hs6[:], t16[:])

                # c3 tail: TC6 = tanh(SC6); SH6 = Go * TC6 — emitted BEFORE
                # the c4 chunk so TC6 sits early in the scalar queue (ready
                # right after the r1 acts, while c4's matmuls run).
                TC6 = wk.tile([128, 2, L6L], bf16, tag="tc6", name="tc6")
                hs5 = wk.tile([128, 2, NL5B], bf16, tag="hs", name="hs5")
                t15 = wk.tile([128, 2, NL5B], bf16, tag="t1", name="t15")
                nc.scalar.activation(TC6[:], SC6[:], Tanh)
                nc.vector.tensor_mul(SH6[:], G[1][:, :, L7P:800], TC6[:])
                nc.vector.tensor_add(t15[:], SH6[:, :, 0:NL5B],
                                     SH6[:, :, NL5B:2 * NL5B])
                nc.vector.tensor_add(hs5[:], SH6[:, :, 2 * NL5B:3 * NL5B],
                                     SH6[:, :, 3 * NL5B:])
                nc.vector.tensor_add(hs5[:], hs5[:], t15[:])

                ps1_cm.__exit__(None, None, None)
                ps2_cm = tc.tile_pool(name="psum2", bufs=1, space="PSUM")
                ps = ps2_cm.__enter__()

                # --- phase 2: internal chunks, c4 (L6-int) then c5 (L5 bulk)
                for (ip, SHc, SCc, goff, GIO, GU, CD, hs, ev) in (
                        (IPMAX, SH7, SC7, 0, GIO6, GU6, C6I, hs6, nc.vector),
                        (NL5B, SH6, SC6, IPMAX, GIO5, GU5, C5V, hs5,
                         nc.vector),
                ):
                    # i/o gates (sigmoid, merged act) + u gate (tanh):
                    # preload gx through the PE (identity matmul, descale in
                    # eye), then accumulate wh @ hs on top. Pfc's preload is
                    # emitted AFTER the wh matmuls: it depends on GX3, which
                    # lands later than hs — keeps the PE queue stall-free.
                    Pio = ps.tile([128, 2, 2, 128], f32, tag=f"pio{ip}",
                                  name=f"pio{ip}")
                    Pu = ps.tile([128, 2, 128], f32, tag=f"pu{ip}",
                                 name=f"pu{ip}")
                    for gi, g in enumerate((0, 1)):
                        nc.tensor.matmul(
                            Pio[:, :, gi, 0:ip], eye_s[:],
                            GXI[g][:, :, goff:goff + ip],
                            start=True, stop=False, skip_group_check=True)
                    nc.tensor.matmul(
                        Pu[:, :, 0:ip], eye_s[:],
                        GXI[2][:, :, goff:goff + ip],
                        start=True, stop=False, skip_group_check=True)
                    Pfc = ps.tile([128, 2, 512], f32, tag=f"pf{ip}",
                                  name=f"pf{ip}")
                    for gi, g in enumerate((0, 1)):
                        for phi in range(2):
                            for k in range(2):
                                nc.tensor.matmul(
                                    Pio[:, phi, gi, 0:ip],
                                    wh_s[:, k, 256 * g + 128 * phi:
                                         256 * g + 128 * phi + 128],
                                    hs[:, k, 0:ip],
                                    start=False, stop=(k == 1),
                                    skip_group_check=True)
                    for phi in range(2):
                        for k in range(2):
                            nc.tensor.matmul(
                                Pu[:, phi, 0:ip],
                                wh_s[:, k, 512 + 128 * phi:
                                     512 + 128 * phi + 128],
                                hs[:, k, 0:ip],
                                start=False, stop=(k == 1),
                                skip_group_check=True)
                    for phi in range(2):
                        gfb = GX3[:, phi, goff:goff + ip][:, None, :]
                        nc.tensor.matmul(
                            Pfc[:, phi, 0:4 * ip], eye_s[:],
                            gfb.broadcast_to([128, K, ip]),
                            start=True, stop=False, skip_group_check=True)
                    for phi in range(2):
                        for k in range(2):
                            nc.tensor.matmul(
                                Pfc[:, phi, 0:4 * ip],
                                whf_s[:, k, 128 * phi:128 * phi + 128],
                                SHc[:, k, 0:4 * ip],
                                start=False, stop=(k == 1),
                                skip_group_check=True)
                    nc.scalar.activation(GIO[:, :, :, 0:ip],
                                         Pio[:, :, :, 0:ip], Sig)
                    nc.scalar.activation(GU[:, :, 0:ip], Pu[:, :, 0:ip], Tanh)
                    # og ships as soon as it exists
                    if ip == IPMAX:
                        nc.sync.dma_start(out=out_h6_d[:],
                                          in_=GIO[:, :, 1, 0:ip])
                    else:
                        nc.sync.dma_start(out=out_h_d[:],
                                          in_=GIO[:, :, 1, 0:ip])

                    # f = sigmoid(gf + Whf @ h_child) per child; gf was
                    # preloaded into Pfc through the PE (descale in eye)
                    FS = wk.tile([128, 2, 4 * ip], bf16, tag="fs",
                                 name=f"fs{ip}")
                    nc.scalar.activation(FS[:], Pfc[:, :, 0:4 * ip], Sig)
                    # c = i*u (runs during the FS activation)
                    ev.tensor_mul(CD[:], GIO[:, :, 0, 0:ip], GU[:, :, 0:ip])
                    ev.tensor_mul(FS[:], FS[:], SCc[:, :, 0:4 * ip])
                    cs1 = wk.tile([128, 2, ip], bf16, tag="cs1",
                                  name=f"cs1{ip}")
                    csum = wk.tile([128, 2, ip], bf16, tag="csum",
                                   name=f"csum{ip}")
                    ev.tensor_add(cs1[:], FS[:, :, 0:ip], FS[:, :, ip:2 * ip])
                    ev.tensor_add(csum[:], FS[:, :, 2 * ip:3 * ip],
                                  FS[:, :, 3 * ip:4 * ip])
                    ev.tensor_add(csum[:], csum[:], cs1[:])
                    ev.tensor_add(CD[:], CD[:], csum[:])
                    if ip == IPMAX:
                        nc.sync.dma_start(out=out_c6_d[:], in_=CD[:])
                    else:
                        nc.sync.dma_start(out=out_c_d[:], in_=CD[:])

            ps2_cm.__exit__(None, None, None)

    nc.compile()
    return nc


def _get_compiled():
    global _COMPILED
    if _COMPILED is None:
        _COMPILED = _build_device_program()
    return _COMPILED


def _numpy_fallback(xs, child_idx, child_mask, emb_table, Wx, bx, Wh, bh,
                    Wout, bout):
    """Exact sequential scan in numpy; only used if the tree is not the
    expected static 4-ary heap."""
    X = np.asarray(emb_table, dtype=F32)[np.asarray(xs)]
    Wx = np.asarray(Wx, dtype=F32)
    Wh = np.asarray(Wh, dtype=F32)
    bx = np.asarray(bx, dtype=F32)
    bh = np.asarray(bh, dtype=F32)
    gx = np.einsum('ghd,nd->ngh', Wx, X).astype(F32) + bx
    Hb = np.zeros((N, H), dtype=F32)
    Cb = np.zeros((N, H), dtype=F32)
    ci = np.asarray(child_idx)
    cm = np.asarray(child_mask, dtype=F32)
    for i in range(N - 1, -1, -1):
        idx = ci[i]
        m = cm[i][:, None]
        Hc = Hb[idx] * m
        Cc = Cb[idx] * m
        hs = Hc.sum(0)
        g = gx[i]
        ig = _sigmoid(g[0] + Wh[0] @ hs + bh[0])
        og = _sigmoid(g[2] + Wh[2] @ hs + bh[2])
        ug = np.tanh(g[3] + Wh[3] @ hs + bh[3]).astype(F32)
        f = _sigmoid(g[1] + Hc @ Wh[1].T + bh[1])
        c = ig * ug + (f * Cc).sum(0)
        Hb[i] = og * np.tanh(c).astype(F32)
        Cb[i] = c
    logits = np.asarray(Wout, dtype=F32) @ Hb[0] + np.asarray(bout, dtype=F32)
    return _log_softmax(logits)


def kernel(xs, child_idx, child_mask, emb_table, Wx, bx, Wh, bh, Wout, bout):
    xs = np.asarray(xs)
    if not (np.array_equal(np.asarray(child_idx), _STATIC_IDX)
            and np.array_equal(np.asarray(child_mask, dtype=F32), _STATIC_MASK)):
        return _numpy_fallback(xs, child_idx, child_mask, emb_table, Wx, bx,
                               Wh, bh, Wout, bout)

    from concourse.bass_utils import run_bass_kernel_spmd

    wx3, wh2, whf2 = _pack_weights(Wx, bx, Wh, bh)
    xts = _pack_xt(xs, emb_table)
    eye = np.ascontiguousarray(np.eye(128, dtype=F32) * DESCALE).astype(BF16)
    in_maps = [
        {"xt": xts[c], "wx": wx3, "wh": wh2, "whf": whf2, "eye": eye}
        for c in range(NCORES)
    ]
    nc = _get_compiled()
    res = run_bass_kernel_spmd(nc, in_maps, core_ids=list(range(NCORES)))

    def _unpack(a):  # [128, 2, n] feature-major halves -> [n, 256]
        a = np.asarray(a, dtype=F32)
        return np.concatenate([a[:, 0, :], a[:, 1, :]], axis=0).T

    Hbuf = np.zeros((1365, H), dtype=F32)
    Cbuf = np.zeros((1365, H), dtype=F32)
    l5h_ids = []     # L5-head nodes (computed on host from L6-head states)
    H6 = []
    C6 = []
    for c in range(NCORES):
        cols, _, l5 = _PLAN[c]
        og5 = _unpack(res.results[c]["out_h"])     # o-gate, not h
        c5v = _unpack(res.results[c]["out_c"])
        Hbuf[l5[24:128]] = og5 * np.tanh(c5v)
        Cbuf[l5[24:128]] = c5v
        l5h_ids.append(l5[0:24])
        og6 = _unpack(res.results[c]["out_h6"])    # o-gate, not h
        c6v = _unpack(res.results[c]["out_c6"])
        H6.append(og6 * np.tanh(c6v))              # [96, 256]
        C6.append(c6v)
    ids = np.concatenate(l5h_ids)                       # [192]
    Hc = np.concatenate(H6).reshape(-1, K, H)           # [192, 4, 256]
    Cc = np.concatenate(C6).reshape(-1, K, H)
    WxF = np.asarray(Wx, dtype=F32)
    WhF = np.asarray(Wh, dtype=F32)
    bxF = np.asarray(bx, dtype=F32)
    bhF = np.asarray(bh, dtype=F32)
    Xl = np.asarray(emb_table, dtype=F32)[xs[ids]]
    gx = np.einsum('ghd,nd->ngh', WxF, Xl).astype(F32) + bxF
    hsum = Hc.sum(1)
    ig = _sigmoid(gx[:, 0] + hsum @ WhF[0].T + bhF[0])
    og = _sigmoid(gx[:, 2] + hsum @ WhF[2].T + bhF[2])
    ug = np.tanh(gx[:, 3] + hsum @ WhF[3].T + bhF[3]).astype(F32)
    f = _sigmoid(gx[:, 1][:, None, :] + Hc @ WhF[1].T + bhF[1])
    cc = ig * ug + (f * Cc).sum(1)
    Hbuf[ids] = og * np.tanh(cc).astype(F32)
    Cbuf[ids] = cc

    h0 = _host_top(Hbuf, Cbuf, xs, emb_table, Wx, bx, Wh, bh)
    logits = np.asarray(Wout, dtype=F32) @ h0 + np.asarray(bout, dtype=F32)
    return _log_softmax(logits)
